# revision 28
# baseline (speedup 1.0000x reference)
"""DenseCapsule routing (2 iterations) on 8 Trainium2 cores.

Sharding: caps_in (C=2048) split across 8 cores (256 each); W-shard +
x-shard stay resident in SBUF, u is recomputed on the fly per c-tile.
Routing state is reduced across cores with two 128KB AllReduces.

Math (ITERATIONS=2, v0=0 => logits after iter1 are 0, cc1 = 1/K):
  u[b,k,c,i]   = sum_j W[k,c,i,j] x[b,c,j]
  v1           = squash(sum_c u / K)
  a[b,k,c]     = sum_i u[b,k,c,i] v1[b,k,i]        (logits for iter 2)
  cc           = softmax_k(a)
  v2           = squash(sum_c cc[b,k,c] u[b,k,c,i])   -> output

All SBUF u/routing tensors use free order (i, k) [i outer, k inner] so
every DVE tensor_tensor op keeps a packed (stride-1, >=2) innermost dim
on every operand -> 2x_1p mode. In particular the cc-broadcast-over-i
multiply gets the stride-0 dim in the middle, not innermost.

Per-core layouts (host-prepped):
  xt  [(c,j)=2048, b=64]            pass-1 lhsT
  wt  [(c,j)=2048, (i,k)=512]       pass-1 rhs & pass-2 u-matmul rhs
  xdo [g=16, (c'16,j8)=128, oct=8, (c16,b8)=128]
      block-diag x: xdo[g,(c'j),o,(c,b)] = x[o*8+b, c0+16g+c', j] * (c==c')
      pass-2 u-matmul lhsT -> psum_u[(c,b), (i,k)] = u[b,k,c,i]
  obd [(c16,b'8)=128, oct=8, b=64]  ones block-diag: delta(b == o*8+b')
      s2 reduction lhsT: psum_s2[b,(i,k)] += sum_c tmp2[(c,b'),(i,k)]

Engine/queue plan (issue order == per-engine execution order):
  SYNC : xt/wt input DMAs, obd, v1d/v1rep replication DMAs, out DMA
  PE   : pass1 (16 mm), u-production (8 mm/g), s2 obd reduction (8 mm/g)
  ACT  : xdo input DMAs, most psum_u->sbuf bf16 copies, exp, squash sqrt
  DVE  : some early psum_u copies, squash chains, routing chain per g
  GPS  : whole AllReduce chains (psum->dram DMA, CC op, dram->sbuf DMA),
         one psum_u copy per g in steady state
"""

import numpy as np

import concourse.bacc as bacc
import concourse.bass as bass
import concourse.tile as tile
from concourse import mybir
from concourse._compat import with_exitstack
from concourse.bass_utils import run_bass_kernel_spmd

NC = 8
B = 64
C = 2048
J = 8
K = 32
I = 16
CL = C // NC        # 256 local caps_in
G = CL // 16        # 16 c-tiles (16 c's each -> 128 (c,j) rows)
KI = K * I          # 512
EPS = 1e-7

F32 = mybir.dt.float32
BF16 = mybir.dt.bfloat16
MULT = mybir.AluOpType.mult
BYPASS = mybir.AluOpType.bypass

TRACE = False           # test.py sets True to capture NTFF timing
LAST_RESULTS = None     # BassKernelResults of the last run

NPRE = 6    # produce() calls issued before the v1 squash block
NPRO = 8    # u-tile lookahead (upool bufs)


def _bcast_last(ap, n):
    """Append a stride-0 dim of size n to an AP (free-dim broadcast)."""
    return bass.AP(tensor=ap.tensor, offset=ap.offset, ap=[*ap.ap, [0, n]])


def _bcast_mid(ap, n):
    """Insert a stride-0 dim of size n before the last free dim, keeping
    the innermost dim packed (preserves DVE 2x_1p eligibility)."""
    return bass.AP(tensor=ap.tensor, offset=ap.offset,
                   ap=[*ap.ap[:-1], [0, n], ap.ap[-1]])


def _squash(nc, pool, eps_t, s_sb, pre, out_ki=False):
    """v = squash(pre * s_sb) for s_sb [B, (i,k)] f32, squash over i.

    squash(s) = (|s|^2 / (1 + |s|^2)) * s / sqrt(|s|^2 + EPS) per (b, k).
    DVE-only except one ACT sqrt. Returns [B, (i,k)] f32 tile, or [B, (k,i)]
    when out_ki (strided write so the output DMA stays contiguous).
    """
    sq = pool.tile([B, KI], F32, tag="sq_sq")
    nc.vector.tensor_mul(sq[:], s_sb[:], s_sb[:])
    n0 = pool.tile([B, K], F32, tag="sq_n0")
    nc.vector.reduce_sum(n0[:], sq[:].rearrange("p (i k) -> p k i", k=K),
                         axis=mybir.AxisListType.X)
    # rt = sqrt(pre^2 * n0 + eps)  -- the one ACT op
    rt = pool.tile([B, K], F32, tag="sq_rt")
    nc.scalar.activation(rt[:], n0[:], mybir.ActivationFunctionType.Sqrt,
                         bias=eps_t[:], scale=pre * pre)
    # dd = rt * (1 + pre^2 n0);  g0 = pre^3 n0 / dd
    srt = pool.tile([B, K], F32, tag="sq_srt")
    nc.vector.scalar_tensor_tensor(out=srt[:], in0=n0[:], scalar=pre * pre,
                                   in1=rt[:], op0=MULT, op1=MULT)
    dd = pool.tile([B, K], F32, tag="sq_dd")
    nc.vector.tensor_add(dd[:], srt[:], rt[:])
    rc = pool.tile([B, K], F32, tag="sq_rc")
    nc.vector.reciprocal(rc[:], dd[:])
    g0 = pool.tile([B, K], F32, tag="sq_g0")
    nc.vector.scalar_tensor_tensor(out=g0[:], in0=n0[:],
                                   scalar=pre * pre * pre,
                                   in1=rc[:], op0=MULT, op1=MULT)
    v = pool.tile([B, KI], F32, tag="sq_v")
    out_ap = (v[:].rearrange("p (k i) -> p i k", i=I) if out_ki
              else v[:].rearrange("p (i k) -> p i k", k=K))
    nc.vector.tensor_mul(out_ap,
                         s_sb[:].rearrange("p (i k) -> p i k", k=K),
                         _bcast_mid(g0[:], I))
    return v


@with_exitstack
def _body(ctx, tc, xt, wt, xdo, obd, out_d):
    nc = tc.nc
    singles = ctx.enter_context(tc.tile_pool(name="singles", bufs=1))
    psA = ctx.enter_context(tc.tile_pool(name="psA", bufs=1, space="PSUM"))
    psU = ctx.enter_context(tc.tile_pool(name="psU", bufs=3, space="PSUM"))
    xpool = ctx.enter_context(tc.tile_pool(name="xpool", bufs=6))
    upool = ctx.enter_context(tc.tile_pool(name="upool", bufs=NPRO))
    work = ctx.enter_context(tc.tile_pool(name="work", bufs=2))
    sm = ctx.enter_context(tc.tile_pool(name="sm", bufs=2))
    dram = ctx.enter_context(tc.tile_pool(name="dram", bufs=1, space="DRAM"))
    ar1_in = dram.tile([B, KI], BF16, name="ar1_in")
    ar1_out = dram.tile([B, KI], BF16, name="ar1_out", addr_space="Shared")
    ar2_in = dram.tile([B, KI], BF16, name="ar2_in")
    ar2_out = dram.tile([B, KI], BF16, name="ar2_out", addr_space="Shared")
    v1d = dram.tile([B, KI], BF16, name="v1d")

    # ---- resident inputs; few big DMAs (SP dispatch is ~650ns/DMA) ----
    xt_sb = singles.tile([128, G, B], BF16)
    wt_sb = singles.tile([128, G, KI], BF16)
    obd_sb = singles.tile([128, 8, B], BF16)
    v1rep = singles.tile([128, 8, KI], BF16)
    nc.sync.dma_start(out=xt_sb[:], in_=xt.rearrange("(g p) b -> p g b", p=128))
    wt3 = wt.rearrange("(g p) n -> p g n", p=128)
    for q in range(4):
        nc.sync.dma_start(out=wt_sb[:, 4 * q:4 * q + 4, :],
                          in_=wt3[:, 4 * q:4 * q + 4, :])
    nc.sync.dma_start(out=obd_sb[:], in_=obd)
    eps_t = singles.tile([B, 1], F32)
    nc.vector.memset(eps_t[:], EPS)

    # ---- pass 1: s1 partial = sum_{c local, j} W x ----
    ps_s1 = psA.tile([B, KI], F32)
    for g in range(G):
        nc.tensor.matmul(ps_s1[:], lhsT=xt_sb[:, g, :],
                         rhs=wt_sb[:, g, :],
                         start=(g == 0), stop=(g == G - 1))
    # AllReduce chain: psum->sbuf on DVE (idle until v1 exists), then the
    # whole dma -> CC -> dma chain on the idle GPSIMD queue. bf16 wire
    # format halves the collective's latency.
    s1p = sm.tile([B, KI], BF16, tag="s1p")
    nc.vector.tensor_copy(s1p[:], ps_s1[:])
    nc.gpsimd.dma_start(out=ar1_in[:], in_=s1p[:])
    nc.gpsimd.collective_compute(
        "AllReduce", mybir.AluOpType.add,
        replica_groups=[list(range(NC))], ins=[ar1_in.opt()], outs=[ar1_out.opt()])
    s1 = sm.tile([B, KI], BF16, tag="s1")
    nc.gpsimd.dma_start(out=s1[:], in_=ar1_out[:])

    # ---- pass 2 producers: u tiles [(c,b), o, (i,k)] via PE + copies ----
    ps_s2 = psA.tile([B, KI], F32)
    nmm = 8 * G
    it = 0
    u_tiles = {}

    def produce(g):
        # xdo arrives via the ACT hw-dge queue (sync queue is busy with
        # xt/wt early on)
        xdo_t = xpool.tile([128, 8, 128], BF16, tag="xdo")
        nc.scalar.dma_start(out=xdo_t[:], in_=xdo[g])
        u_g = upool.tile([128, 8, KI], BF16, name=f"ug{g}", tag="ug")
        for op in range(4):
            ps_u = psU.tile([128, 2, KI], F32, tag="psu")
            for h in range(2):
                nc.tensor.matmul(ps_u[:, h, :], lhsT=xdo_t[:, 2 * op + h, :],
                                 rhs=wt_sb[:, g, :],
                                 start=True, stop=True)
            # psum -> sbuf bf16 copy (ACT; GPSIMD cannot read PSUM).
            # ACT paces produce at ~4.9us/g, under the ~7.3us/g consume.
            nc.scalar.copy(u_g[:, 2 * op:2 * op + 2, :], ps_u[:])
        u_tiles[g] = u_g

    cstate = {}

    def consume_front(g):
        # a[b,k,c] = sum_i u * v1: mul + 4-level tree on DVE, exp on ACT.
        # The softmax denominator + everything after it live in
        # consume_back, emitted AFTER front(g+1) on the DVE queue, so DVE
        # never waits on the ACT exp round-trip.
        u_g = u_tiles[g]
        tmp = work.tile([128, 8, KI], BF16, tag="tmp")
        nc.vector.tensor_mul(tmp[:], u_g[:], v1rep[:])
        t4 = tmp[:].rearrange("p o (i k) -> p o i k", k=K)
        f1 = work.tile([128, 8, 8, K], BF16, tag="f1")
        nc.vector.tensor_add(f1[:], t4[:, :, 0:8, :], t4[:, :, 8:16, :])
        f2 = sm.tile([128, 8, 4, K], BF16, tag="f2")
        nc.vector.tensor_add(f2[:], f1[:, :, 0:4, :], f1[:, :, 4:8, :])
        f3 = sm.tile([128, 8, 2, K], BF16, tag="f3")
        nc.vector.tensor_add(f3[:], f2[:, :, 0:2, :], f2[:, :, 2:4, :])
        a_t = sm.tile([128, 8, K], F32, tag="a")
        nc.vector.tensor_add(a_t[:], f3[:, :, 0, :], f3[:, :, 1, :])
        e_t = sm.tile([128, 8, K], BF16, tag="e")
        nc.scalar.activation(e_t[:], a_t[:],
                             mybir.ActivationFunctionType.Exp, scale=1.0)
        cstate[g] = e_t

    def consume_back(g):
        nonlocal it
        u_g = u_tiles.pop(g)
        e_t = cstate.pop(g)
        den = sm.tile([128, 8], F32, tag="den")
        nc.vector.reduce_sum(den[:], e_t[:], axis=mybir.AxisListType.X)
        rcp = sm.tile([128, 8], F32, tag="rcp")
        nc.vector.reciprocal(rcp[:], den[:])
        cc = sm.tile([128, 8, K], BF16, tag="cc")
        nc.vector.tensor_mul(cc[:], e_t[:], _bcast_last(rcp[:], K))
        # tmp2 = u * cc (broadcast over i sits mid-AP; k stays packed)
        tmp2 = work.tile([128, 8, KI], BF16, tag="tmp2")
        nc.vector.tensor_mul(
            tmp2[:].rearrange("p o (i k) -> p o i k", k=K),
            u_g[:].rearrange("p o (i k) -> p o i k", k=K),
            _bcast_mid(cc[:], I))
        for o in range(8):
            nc.tensor.matmul(ps_s2[:], lhsT=obd_sb[:, o, :],
                             rhs=tmp2[:, o, :], start=(it == 0),
                             stop=(it == nmm - 1))
            it += 1

    # Pre-issue exactly NPRE produces: ACT reaches the squash sqrt and
    # exp(0) right as s1/a_t(0) land; later produces are issued inside the
    # consume loop AFTER each consume so exp(g) isn't queued behind them.
    for g in range(NPRE):
        produce(g)

    # ---- v1 = squash(s1/K); replicate across (c,b) partitions ----
    v1 = _squash(nc, sm, eps_t, s1, 1.0 / K)
    v1b = sm.tile([B, KI], BF16, tag="v1b")
    nc.vector.tensor_copy(v1b[:], v1[:])
    # replication DMAs: split across sync + scalar queues so the 8 octant
    # dispatches (~650ns each) serialize over two engines, not one
    nc.sync.dma_start(out=v1d[:], in_=v1b[:])
    v1d_ap = v1d[:]
    for o in range(8):
        src_ap = bass.AP(tensor=v1d_ap.tensor,
                         offset=v1d_ap.offset + o * 8 * KI,
                         ap=[[0, 16], [KI, 8], [1, KI]])
        eng = nc.sync if o % 2 == 0 else nc.scalar
        eng.dma_start(out=v1rep[:, o, :], in_=src_ap)

    consume_front(0)
    for g in range(G):
        if g + 1 < G:
            consume_front(g + 1)
        consume_back(g)
        if g + NPRE < G:
            produce(g + NPRE)

    # ---- AllReduce s2, v2 = squash(s2), store out ----
    s2p = sm.tile([B, KI], BF16, tag="s2p")
    nc.vector.tensor_copy(s2p[:], ps_s2[:])
    nc.gpsimd.dma_start(out=ar2_in[:], in_=s2p[:])
    nc.gpsimd.collective_compute(
        "AllReduce", mybir.AluOpType.add,
        replica_groups=[list(range(NC))], ins=[ar2_in.opt()], outs=[ar2_out.opt()])
    s2 = sm.tile([B, KI], BF16, tag="s2")
    nc.gpsimd.dma_start(out=s2[:], in_=ar2_out[:])
    v2 = _squash(nc, sm, eps_t, s2, 1.0, out_ki=True)
    nc.sync.dma_start(out=out_d, in_=v2[:])


_PROG = None


def _get_program():
    global _PROG
    if _PROG is None:
        nc = bacc.Bacc("TRN2", target_bir_lowering=False, debug=False,
                       num_devices=NC)
        xt_d = nc.dram_tensor("xt", [CL * J, B], BF16, kind="ExternalInput")
        wt_d = nc.dram_tensor("wt", [CL * J, KI], BF16, kind="ExternalInput")
        xdo_d = nc.dram_tensor("xdo", [G, 128, 8, 128], BF16,
                               kind="ExternalInput")
        obd_d = nc.dram_tensor("obd", [128, 8, B], BF16, kind="ExternalInput")
        out_d = nc.dram_tensor("out", [B, KI], F32, kind="ExternalOutput")
        with tile.TileContext(nc) as tc:
            _body(tc, xt_d[:], wt_d[:], xdo_d[:], obd_d[:], out_d[:])
        nc.compile()
        _PROG = nc
    return _PROG


def _constant_mats():
    import ml_dtypes
    obd = np.zeros((16, 8, 8, B), np.float32)       # [c, b', oct, b]
    for o in range(8):
        for bp in range(8):
            obd[:, bp, o, o * 8 + bp] = 1.0
    obd = obd.reshape(128, 8, B).astype(ml_dtypes.bfloat16)
    return obd


def kernel(x, W):
    global LAST_RESULTS
    x = np.ascontiguousarray(np.asarray(x, np.float32))
    W = np.ascontiguousarray(np.asarray(W, np.float32))
    assert x.shape == (B, C, J) and W.shape == (K, C, I, J)
    nc = _get_program()
    obd = _constant_mats()
    in_maps = []
    for m in range(NC):
        xs = x[:, m * CL:(m + 1) * CL, :]                       # [B, CL, J]
        Ws = W[:, m * CL:(m + 1) * CL, :, :]                    # [K, CL, I, J]
        import ml_dtypes
        bf = ml_dtypes.bfloat16
        xt = np.ascontiguousarray(
            xs.transpose(1, 2, 0)).reshape(CL * J, B).astype(bf)
        wt = np.ascontiguousarray(
            Ws.transpose(1, 3, 2, 0)).reshape(CL * J, I * K).astype(bf)
        A = xs.reshape(8, 8, G, 16, J)                          # [o, b, g, c', j]
        xdo = np.zeros((G, 16, J, 8, 16, 8), np.float32)        # [g,c',j,o,c,b]
        for cp in range(16):
            xdo[:, cp, :, :, cp, :] = A[:, :, :, cp, :].transpose(2, 3, 0, 1)
        xdo = xdo.reshape(G, 128, 8, 128).astype(bf)
        in_maps.append({"xt": xt, "wt": wt, "xdo": xdo, "obd": obd})
    res = run_bass_kernel_spmd(nc, in_maps, core_ids=list(range(NC)),
                               trace=TRACE)
    LAST_RESULTS = res
    return np.asarray(res.results[0]["out"], np.float32).reshape(B, K, I)
